# revision 29
# baseline (speedup 1.0000x reference)
"""DGL MPNN layer on 8 Trainium2 NeuronCores — Khatri-Rao edge pipeline.

Math (per reference):
    w_e  = (ef_e @ We + be).reshape(32, 32)          # per-edge weight
    msg_e = nf[src_e] @ w_e                          # (32,)
    out_n = sum_{e: dst_e==n} msg_e + nf_n + bias

Identity used on device:
    msg[e,o] = sum_{d,i} ef[e,d]*x[e,i]*We[d,32i+o] + sum_i x[e,i]*be[32i+o]
             = sum_g  Wbig_g^T @ (xsrcT4 ⊙ efrep_g)[:, e]  +  BeW^T @ x^T[:, e]

Device pipeline per 2048-edge granule (16 chunks of 128 edges), tiles use
a (8 d x 16 i) partition layout so the feed needs only 2 ef-tiles + 2
x-planes (replication minimized):
    xs_t  <- linear DMA of host-pregathered x^T half-planes (8x rep, f16)
    efr   <- replicating-AP DMAs from eft HBM (ef^T rows expanded 16x)
    y     <- DVE f16 mult (2x_1P mode), in1 broadcast over the plane pair
    sel   <- DMA of host-built one-hot scatter blocks
  per chunk:
    msg    <- PE: 4 matmuls lhsT=y slice rhs=Wbig_t + 2 K=16 BeW matmuls
    msg_sb <- ACT batched copy PSUM->SBUF f16 (16 chunks per op)
    acc    <- PE scatter lhsT=sel chunk rhs=msg_sb accumulating directly in
              a PSUM-resident accumulator (init per tile = identity-matmul
              of the f16 node table + ones x bias K=1 matmul)
    out    <- ACT PSUM->SBUF f16 staging per 10 tiles + DMA

Sharding: edges partitioned by dst node range (6250 nodes/core); nodes
LPT-balanced into NT=50 tiles of <=128 nodes / <=CPT*128 edges so every
tile owns exactly CPT chunks (SPMD-uniform control flow; pad slots have
all-zero sel columns). Host inverse-permutes the output rows.
"""

import numpy as np

N, E, HID, ED = 50000, 200000, 32, 16
NCORES = 8
NPC = N // NCORES            # 6250 nodes per core
NT = 50                      # node tiles per core (load-balanced, 128 nodes max)
NPC_PAD = NT * 128
GRAN = 2048                  # edges per granule
CH = GRAN // 128             # chunks per granule


def _balance(dl):
    """LPT-assign local nodes to NT tiles (<=128 nodes each), balancing edge
    load. Returns (tile_of, pos_of) for each local node and per-tile loads."""
    import heapq

    deg = np.bincount(dl, minlength=NPC)
    order = np.argsort(-deg, kind="stable")
    heap = [(0, a) for a in range(NT)]
    heapq.heapify(heap)
    counts = np.zeros(NT, np.int64)
    tile_of = np.zeros(NPC, np.int64)
    pos_of = np.zeros(NPC, np.int64)
    spill = []
    for n in order:
        load, a = heapq.heappop(heap)
        tile_of[n] = a
        pos_of[n] = counts[a]
        counts[a] += 1
        load += int(deg[n])
        if counts[a] < 128:
            heapq.heappush(heap, (load, a))
        else:
            spill.append((load, a))
    loads = np.zeros(NT, np.int64)
    np.add.at(loads, tile_of, deg)
    return tile_of, pos_of, loads


def _prep(nf, initial_ef, src, dst, We, be, bias):
    nf = np.ascontiguousarray(np.asarray(nf, dtype=np.float32))
    ef = np.ascontiguousarray(np.asarray(initial_ef, dtype=np.float32))
    src = np.asarray(src).astype(np.int64)
    dst = np.asarray(dst).astype(np.int64)
    We = np.asarray(We, dtype=np.float32)
    be = np.asarray(be, dtype=np.float32)
    bias = np.asarray(bias, dtype=np.float32)

    # Wbig block t (t=2g+ih): rows p=16*dd+iw map to (d=8g+dd, i=16*ih+iw)
    W3 = We.reshape(ED, HID, HID)                      # [d, i, o]
    wbig = np.zeros((128, 4 * HID), np.float16)
    p_dd, p_iw = np.arange(128) // 16, np.arange(128) % 16
    for t in range(4):
        g, ih = t // 2, t % 2
        wbig[:, 32 * t:32 * t + 32] = \
            W3[8 * g + p_dd, 16 * ih + p_iw, :].astype(np.float16)
    bew = np.ascontiguousarray(be.reshape(2, 16, HID).transpose(1, 0, 2)).astype(np.float16)  # [iw, ih, o]

    core_of = dst // NPC
    cores = []
    cpt_max = 1
    for c in range(NCORES):
        eidx = np.nonzero(core_of == c)[0]
        dl = (dst[eidx] - c * NPC).astype(np.int64)
        tile_of, pos_of, loads = _balance(dl)
        cpt_max = max(cpt_max, int(np.ceil(loads.max() / 128)))
        cores.append((eidx, dl, tile_of, pos_of, loads))

    CPT = cpt_max
    n_chunks = NT * CPT
    E_pad = ((n_chunks * 128 + GRAN - 1) // GRAN) * GRAN

    nfh = nf.astype(np.float16)
    efh = ef.astype(np.float16)

    in_maps = []
    perms = []
    for c, (eidx, dl, tile_of, pos_of, loads) in enumerate(cores):
        # order edges by tile; tile a owns slots [a*CPT*128, (a+1)*CPT*128)
        et = tile_of[dl]
        order = np.argsort(et, kind="stable")
        eidx = eidx[order]
        dl = dl[order]
        et = et[order]

        slot_src = np.zeros(E_pad, np.int64)
        slot_ef = np.zeros((E_pad, ED), np.float16)
        dstl = np.zeros(E_pad, np.int64)
        valid = np.zeros(E_pad, bool)
        pos = 0
        for a in range(NT):
            n_a = int(loads[a])
            s0 = a * CPT * 128
            sl = slice(pos, pos + n_a)
            slot_src[s0:s0 + n_a] = src[eidx[sl]]
            slot_ef[s0:s0 + n_a] = efh[eidx[sl]]
            dstl[s0:s0 + n_a] = pos_of[dl[sl]]
            valid[s0:s0 + n_a] = True
            pos += n_a

        # one-hot scatter matrix, chunk-blocked: sel[e, 128*c + n]
        slots = np.nonzero(valid)[0]
        sel_h = np.zeros((128, E_pad), np.float16)
        sel_h[slots % 128, 128 * (slots // 128) + dstl[slots]] = 1.0

        # x^T half-planes, each replicated 8x along partitions: (128,2,E_pad)
        xs = np.zeros((E_pad, HID), np.float16)
        xs[valid] = nfh[slot_src[valid]]
        xsrcT4 = np.stack([np.tile(xs.T[0:16], (8, 1)),
                           np.tile(xs.T[16:32], (8, 1))], axis=1)

        eft = np.ascontiguousarray(slot_ef.T)          # (16, E_pad)

        # node table permuted into (tile, pos) order: (128, NT, HID) f16
        rows = tile_of * 128 + pos_of                  # device row per local node
        nfb = np.zeros((NPC_PAD, HID), np.float16)
        nfb[rows] = nfh[c * NPC:(c + 1) * NPC]
        nfb = np.ascontiguousarray(
            nfb.reshape(NT, 128, HID).transpose(1, 0, 2))
        perms.append(rows)

        in_maps.append({
            "xsrc": np.ascontiguousarray(xsrcT4),
            "eft": eft,
            "selh": sel_h,
            "ident": np.eye(128, dtype=np.float16),
            "wbig": wbig,
            "bew": bew,
            "nfb": nfb,
            "bias1": bias.reshape(1, HID).astype(np.float16),
            "ones1": np.ones((1, 128), np.float16),
        })
    return in_maps, CPT, E_pad, perms


def build_nc(CPT, E_pad, U_pad=0):
    import concourse.bass as bass
    import concourse.bacc as bacc
    import concourse.mybir as mybir
    import concourse.tile as tile
    import os

    f16 = mybir.dt.float16
    f32 = mybir.dt.float32

    G = E_pad // GRAN
    n_chunks = NT * CPT
    kmax = int(os.environ.get("KMAX_CHUNKS", "0"))
    if kmax:
        n_chunks = min(n_chunks, kmax)
        G = min(G, (n_chunks * 128 + GRAN - 1) // GRAN)

    nc = bacc.Bacc("TRN2", target_bir_lowering=False, debug=False)
    xsrc = nc.dram_tensor("xsrc", [128, 2, E_pad], f16, kind="ExternalInput")
    eft = nc.dram_tensor("eft", [16, E_pad], f16, kind="ExternalInput")
    selh = nc.dram_tensor("selh", [128, E_pad], f16, kind="ExternalInput")
    ident = nc.dram_tensor("ident", [128, 128], f16, kind="ExternalInput")
    wbig = nc.dram_tensor("wbig", [128, 4 * HID], f16, kind="ExternalInput")
    bew = nc.dram_tensor("bew", [16, 2, HID], f16, kind="ExternalInput")
    nfb = nc.dram_tensor("nfb", [128, NT, HID], f16, kind="ExternalInput")
    bias1 = nc.dram_tensor("bias1", [1, HID], f16, kind="ExternalInput")
    ones1 = nc.dram_tensor("ones1", [1, 128], f16, kind="ExternalInput")
    out = nc.dram_tensor("out", [NPC_PAD, HID], f16, kind="ExternalOutput")

    OUT_BLK = 10  # tiles per PSUM->SBUF output copy + DMA burst
    MB = 16       # chunks per batched msg copy (2 KB PSUM bank)

    with tile.TileContext(nc) as tc:
        with (
            tc.tile_pool(name="const", bufs=1) as cpool,
            tc.tile_pool(name="xt", bufs=4) as xt_pool,
            tc.tile_pool(name="efr", bufs=4) as efr_pool,
            tc.tile_pool(name="yy", bufs=3) as y_pool,
            tc.tile_pool(name="sel", bufs=4) as sel_pool,
            tc.tile_pool(name="msg", bufs=4) as msg_pool,
            tc.tile_pool(name="ob", bufs=2) as ob_pool,
            tc.tile_pool(name="mps", bufs=2, space="PSUM") as mps_pool,
            tc.tile_pool(name="acc", bufs=1, space="PSUM") as acc_pool,
        ):
            ident_sb = cpool.tile([128, 128], f16)
            wbig_sb = cpool.tile([128, 4 * HID], f16)
            bew_sb = cpool.tile([16, 2, HID], f16)
            ones_sb = cpool.tile([1, 128], f16)
            bias_sb = cpool.tile([1, HID], f16)
            nfb_sb = cpool.tile([128, NT, HID], f16)

            nc.gpsimd.dma_start(wbig_sb[:], wbig[:])
            nc.gpsimd.dma_start(bew_sb[:], bew[:])
            nc.gpsimd.dma_start(ones_sb[:], ones1[:])
            nc.gpsimd.dma_start(bias_sb[:], bias1[:])
            nc.gpsimd.dma_start(ident_sb[:], ident[:])
            nc.gpsimd.dma_start(nfb_sb[:], nfb[:])

            acc = acc_pool.tile([128, NT, HID], f32)
            out_r = out[:].rearrange("(a p) c -> p a c", p=128)
            for g in range(G):
                e0 = g * GRAN
                gw = min(GRAN, n_chunks * 128 - e0)
                xs_t = xt_pool.tile([128, 2, GRAN], f16, tag="xs")
                H = gw // 2
                xengs = (nc.sync, nc.scalar, nc.scalar, nc.sync)
                for ih in range(2):
                    for h in range(2):
                        xengs[2 * ih + h].dma_start(
                            xs_t[:, ih, h * H:h * H + H],
                            xsrc[:, ih, e0 + h * H:e0 + (h + 1) * H])
                efr = efr_pool.tile([128, 2, GRAN], f16, tag="efr")
                ea = eft[:]
                engs_map = {"a": nc.scalar, "s": nc.sync, "p": nc.gpsimd}
                engs = tuple(engs_map[ch] for ch in
                             os.environ.get("EFR_ENGS", "asps"))
                for g2 in range(2):
                    for h in range(2):
                        # efrep[16*dd+iw, e] = eft[8*g2+dd, e0+e]
                        rep = bass.AP(ea.tensor,
                                      ea.offset + 8 * g2 * E_pad + e0 + h * H,
                                      [[E_pad, 8], [0, 16], [1, H]])
                        engs[2 * g2 + h].dma_start(efr[:, g2, h * H:h * H + H],
                                                   rep)

                sel_t = sel_pool.tile([128, CH, 128], f16, tag="sel")
                nc.gpsimd.dma_start(
                    sel_t[:, 0:gw // 128, :],
                    selh[:, e0:e0 + gw]
                    .rearrange("p (c n) -> p c n", n=128))

                y_t = y_pool.tile([128, 4, GRAN], f16, tag="y")
                with nc.allow_low_precision("f16 products, tol loose"):
                    for g2 in range(2):
                        ef1 = efr[:, g2, :]
                        ef_bc = bass.AP(ef1.tensor, ef1.offset,
                                        [ef1.ap[0], [0, 2], [1, gw]])
                        nc.vector.tensor_tensor(
                            out=y_t[:, 2 * g2:2 * g2 + 2, 0:gw],
                            in0=xs_t[:, :, 0:gw],
                            in1=ef_bc, op=mybir.AluOpType.mult)

                for cb in range(0, CH, MB):
                    cis = [ci for ci in range(cb, min(cb + MB, CH))
                           if g * CH + ci < n_chunks]
                    if not cis:
                        break
                    nb = len(cis)
                    msg_ps = mps_pool.tile([128, MB, HID], f32, tag="msgp")
                    for j, ci in enumerate(cis):
                        sl = slice(128 * ci, 128 * ci + 128)
                        for t in range(4):
                            nc.tensor.matmul(
                                msg_ps[:, j, :], y_t[:, t, sl],
                                wbig_sb[:, 32 * t:32 * t + 32],
                                start=(t == 0), stop=False)
                        nc.tensor.matmul(msg_ps[:, j, :], xs_t[0:16, 0, sl],
                                         bew_sb[:, 0, :], start=False,
                                         stop=False)
                        nc.tensor.matmul(msg_ps[:, j, :], xs_t[0:16, 1, sl],
                                         bew_sb[:, 1, :], start=False,
                                         stop=True)
                    msg_sb = msg_pool.tile([128, MB, HID], f16, tag="msgs")
                    with nc.allow_low_precision("accumulated in f32 PSUM"):
                        nc.scalar.copy(msg_sb[:, 0:nb, :], msg_ps[:, 0:nb, :])
                    for j, ci in enumerate(cis):
                        c = g * CH + ci
                        a, k = c // CPT, c % CPT
                        if k == 0:
                            nc.tensor.matmul(acc[:, a, :], ident_sb[:],
                                             nfb_sb[:, a, :],
                                             start=True, stop=False)
                            nc.tensor.matmul(acc[:, a, :], ones_sb[:],
                                             bias_sb[:], start=False,
                                             stop=False)
                        nc.tensor.matmul(acc[:, a, :], sel_t[:, ci, :],
                                         msg_sb[:, j, :],
                                         start=False, stop=(k == CPT - 1))
                        if k == CPT - 1 and (a + 1) % OUT_BLK == 0:
                            b0 = a + 1 - OUT_BLK
                            ob = ob_pool.tile([128, OUT_BLK, HID], f16,
                                              tag="ob")
                            with nc.allow_low_precision("f16 out staging"):
                                nc.scalar.copy(ob[:], acc[:, b0:a + 1, :])
                            nc.sync.dma_start(out_r[:, b0:a + 1, :], ob[:])

            if n_chunks == NT * CPT:
                b0 = (NT // OUT_BLK) * OUT_BLK
                if b0 < NT:
                    ob = ob_pool.tile([128, NT - b0, HID], f16, name="obt")
                    with nc.allow_low_precision("f16 out staging"):
                        nc.scalar.copy(ob[:], acc[:, b0:NT, :])
                    nc.sync.dma_start(out_r[:, b0:NT, :], ob[:])
    nc.compile()
    return nc


_CACHE = {}


def kernel(nf, initial_ef, src, dst, We, be, bias):
    in_maps, CPT, E_pad, perms = _prep(nf, initial_ef, src, dst, We, be, bias)
    key = (CPT, E_pad)
    if key not in _CACHE:
        _CACHE[key] = build_nc(CPT, E_pad)
    nc = _CACHE[key]

    from concourse.bass_utils import run_bass_kernel_spmd
    res = run_bass_kernel_spmd(nc, in_maps, core_ids=list(range(NCORES)))
    outs = [r["out"][perms[c], :HID] for c, r in enumerate(res.results)]
    return np.ascontiguousarray(np.concatenate(outs, axis=0).astype(np.float32))


# revision 31
# speedup vs baseline: 1.2655x; 1.2655x over previous
"""DGL MPNN layer on 8 Trainium2 NeuronCores — Khatri-Rao edge pipeline.

Math (per reference):
    w_e  = (ef_e @ We + be).reshape(32, 32)          # per-edge weight
    msg_e = nf[src_e] @ w_e                          # (32,)
    out_n = sum_{e: dst_e==n} msg_e + nf_n + bias

Identity used on device:
    msg[e,o] = sum_{d,i} ef[e,d]*x[e,i]*We[d,32i+o] + sum_i x[e,i]*be[32i+o]
             = sum_g  Wbig_g^T @ (xsrcT4 ⊙ efrep_g)[:, e]  +  BeW^T @ x^T[:, e]

Device pipeline per 2048-edge granule (16 chunks of 128 edges), tiles use
a (8 d x 16 i) partition layout so the feed needs only 2 ef-tiles + 2
x-planes (replication minimized):
    xs_t  <- linear DMA of host-pregathered x^T half-planes (8x rep, f16)
    efr   <- replicating-AP DMAs from eft HBM (ef^T rows expanded 16x)
    y     <- DVE f16 mult (2x_1P mode), in1 broadcast over the plane pair
    sel   <- DMA of host-built one-hot scatter blocks
  per chunk:
    msg    <- PE: 4 matmuls lhsT=y slice rhs=Wbig_t + 2 K=16 BeW matmuls
    msg_sb <- ACT batched copy PSUM->SBUF f16 (16 chunks per op)
    acc    <- PE scatter lhsT=sel chunk rhs=msg_sb accumulating directly in
              a PSUM-resident accumulator (init per tile = identity-matmul
              of the f16 node table + ones x bias K=1 matmul)
    out    <- ACT PSUM->SBUF f16 staging per 10 tiles + DMA

Sharding: edges partitioned by dst node range (6250 nodes/core); nodes
LPT-balanced into NT=50 tiles of <=128 nodes / <=CPT*128 edges so every
tile owns exactly CPT chunks (SPMD-uniform control flow; pad slots have
all-zero sel columns). Host inverse-permutes the output rows.
"""

import numpy as np

N, E, HID, ED = 50000, 200000, 32, 16
NCORES = 8
NPC = N // NCORES            # 6250 nodes per core
NT = 50                      # node tiles per core (load-balanced, 128 nodes max)
NPC_PAD = NT * 128
GRAN = 2048                  # edges per granule
CH = GRAN // 128             # chunks per granule


def _balance(dl):
    """LPT-assign local nodes to NT tiles (<=128 nodes each), balancing edge
    load. Returns (tile_of, pos_of) for each local node and per-tile loads."""
    import heapq

    deg = np.bincount(dl, minlength=NPC)
    order = np.argsort(-deg, kind="stable")
    heap = [(0, a) for a in range(NT)]
    heapq.heapify(heap)
    counts = np.zeros(NT, np.int64)
    tile_of = np.zeros(NPC, np.int64)
    pos_of = np.zeros(NPC, np.int64)
    spill = []
    for n in order:
        load, a = heapq.heappop(heap)
        tile_of[n] = a
        pos_of[n] = counts[a]
        counts[a] += 1
        load += int(deg[n])
        if counts[a] < 128:
            heapq.heappush(heap, (load, a))
        else:
            spill.append((load, a))
    loads = np.zeros(NT, np.int64)
    np.add.at(loads, tile_of, deg)
    return tile_of, pos_of, loads


def _prep(nf, initial_ef, src, dst, We, be, bias):
    nf = np.ascontiguousarray(np.asarray(nf, dtype=np.float32))
    ef = np.ascontiguousarray(np.asarray(initial_ef, dtype=np.float32))
    src = np.asarray(src).astype(np.int64)
    dst = np.asarray(dst).astype(np.int64)
    We = np.asarray(We, dtype=np.float32)
    be = np.asarray(be, dtype=np.float32)
    bias = np.asarray(bias, dtype=np.float32)

    # Wbig block t (t=2g+ih): rows p=16*dd+iw map to (d=8g+dd, i=16*ih+iw)
    W3 = We.reshape(ED, HID, HID)                      # [d, i, o]
    wbig = np.zeros((128, 4 * HID), np.float16)
    p_dd, p_iw = np.arange(128) // 16, np.arange(128) % 16
    for t in range(4):
        g, ih = t // 2, t % 2
        wbig[:, 32 * t:32 * t + 32] = \
            W3[8 * g + p_dd, 16 * ih + p_iw, :].astype(np.float16)
    bew = np.ascontiguousarray(be.reshape(2, 16, HID).transpose(1, 0, 2)).astype(np.float16)  # [iw, ih, o]

    core_of = dst // NPC
    cores = []
    cpt_max = 1
    for c in range(NCORES):
        eidx = np.nonzero(core_of == c)[0]
        dl = (dst[eidx] - c * NPC).astype(np.int64)
        tile_of, pos_of, loads = _balance(dl)
        cpt_max = max(cpt_max, int(np.ceil(loads.max() / 128)))
        cores.append((eidx, dl, tile_of, pos_of, loads))

    CPT = cpt_max
    n_chunks = NT * CPT
    E_pad = ((n_chunks * 128 + GRAN - 1) // GRAN) * GRAN

    nfh = nf.astype(np.float16)
    efh = ef.astype(np.float16)

    in_maps = []
    perms = []
    for c, (eidx, dl, tile_of, pos_of, loads) in enumerate(cores):
        # order edges by tile; tile a owns slots [a*CPT*128, (a+1)*CPT*128)
        et = tile_of[dl]
        order = np.argsort(et, kind="stable")
        eidx = eidx[order]
        dl = dl[order]
        et = et[order]

        slot_src = np.zeros(E_pad, np.int64)
        slot_ef = np.zeros((E_pad, ED), np.float16)
        dstl = np.zeros(E_pad, np.int64)
        valid = np.zeros(E_pad, bool)
        pos = 0
        for a in range(NT):
            n_a = int(loads[a])
            s0 = a * CPT * 128
            sl = slice(pos, pos + n_a)
            slot_src[s0:s0 + n_a] = src[eidx[sl]]
            slot_ef[s0:s0 + n_a] = efh[eidx[sl]]
            dstl[s0:s0 + n_a] = pos_of[dl[sl]]
            valid[s0:s0 + n_a] = True
            pos += n_a

        # one-hot scatter matrix, chunk-blocked: sel[e, 128*c + n]
        slots = np.nonzero(valid)[0]
        sel_h = np.zeros((128, E_pad), np.float16)
        sel_h[slots % 128, 128 * (slots // 128) + dstl[slots]] = 1.0

        # x^T half-planes, each replicated 8x along partitions: (128,2,E_pad)
        xs = np.zeros((E_pad, HID), np.float16)
        xs[valid] = nfh[slot_src[valid]]
        xsrcT4 = np.stack([np.tile(xs.T[0:16], (8, 1)),
                           np.tile(xs.T[16:32], (8, 1))], axis=1)

        eft = np.ascontiguousarray(slot_ef.T)          # (16, E_pad)

        # node table permuted into (tile, pos) order: (128, NT, HID) f16
        rows = tile_of * 128 + pos_of                  # device row per local node
        nfb = np.zeros((NPC_PAD, HID), np.float16)
        nfb[rows] = nfh[c * NPC:(c + 1) * NPC]
        nfb = np.ascontiguousarray(
            nfb.reshape(NT, 128, HID).transpose(1, 0, 2))
        perms.append(rows)

        in_maps.append({
            "xsrc": np.ascontiguousarray(xsrcT4),
            "eft": eft,
            "selh": sel_h,
            "ident": np.eye(128, dtype=np.float16),
            "wbig": wbig,
            "bew": bew,
            "nfb": nfb,
            "bias1": bias.reshape(1, HID).astype(np.float16),
            "ones1": np.ones((1, 128), np.float16),
        })
    return in_maps, CPT, E_pad, perms


def build_nc(CPT, E_pad, U_pad=0):
    import concourse.bass as bass
    import concourse.bacc as bacc
    import concourse.mybir as mybir
    import concourse.tile as tile
    import os

    f16 = mybir.dt.float16
    f32 = mybir.dt.float32

    G = E_pad // GRAN
    n_chunks = NT * CPT
    kmax = int(os.environ.get("KMAX_CHUNKS", "0"))
    if kmax:
        n_chunks = min(n_chunks, kmax)
        G = min(G, (n_chunks * 128 + GRAN - 1) // GRAN)

    nc = bacc.Bacc("TRN2", target_bir_lowering=False, debug=False)
    xsrc = nc.dram_tensor("xsrc", [128, 2, E_pad], f16, kind="ExternalInput")
    eft = nc.dram_tensor("eft", [16, E_pad], f16, kind="ExternalInput")
    selh = nc.dram_tensor("selh", [128, E_pad], f16, kind="ExternalInput")
    ident = nc.dram_tensor("ident", [128, 128], f16, kind="ExternalInput")
    wbig = nc.dram_tensor("wbig", [128, 4 * HID], f16, kind="ExternalInput")
    bew = nc.dram_tensor("bew", [16, 2, HID], f16, kind="ExternalInput")
    nfb = nc.dram_tensor("nfb", [128, NT, HID], f16, kind="ExternalInput")
    bias1 = nc.dram_tensor("bias1", [1, HID], f16, kind="ExternalInput")
    ones1 = nc.dram_tensor("ones1", [1, 128], f16, kind="ExternalInput")
    out = nc.dram_tensor("out", [NPC_PAD, HID], f16, kind="ExternalOutput")

    OUT_BLK = 10  # tiles per PSUM->SBUF output copy + DMA burst
    MB = 16       # chunks per batched msg copy (2 KB PSUM bank)

    with tile.TileContext(nc) as tc:
        with (
            tc.tile_pool(name="const", bufs=1) as cpool,
            tc.tile_pool(name="xt", bufs=4) as xt_pool,
            tc.tile_pool(name="efr", bufs=4) as efr_pool,
            tc.tile_pool(name="yy", bufs=3) as y_pool,
            tc.tile_pool(name="sel", bufs=4) as sel_pool,
            tc.tile_pool(name="msg", bufs=4) as msg_pool,
            tc.tile_pool(name="ob", bufs=2) as ob_pool,
            tc.tile_pool(name="mps", bufs=2, space="PSUM") as mps_pool,
            tc.tile_pool(name="acc", bufs=1, space="PSUM") as acc_pool,
        ):
            ident_sb = cpool.tile([128, 128], f16)
            wbig_sb = cpool.tile([128, 4 * HID], f16)
            bew_sb = cpool.tile([16, 2, HID], f16)
            ones_sb = cpool.tile([1, 128], f16)
            bias_sb = cpool.tile([1, HID], f16)
            nfb_sb = cpool.tile([128, NT, HID], f16)

            def emit_consts():
                nc.gpsimd.dma_start(wbig_sb[:], wbig[:])
                nc.gpsimd.dma_start(bew_sb[:], bew[:])
                nc.gpsimd.dma_start(ones_sb[:], ones1[:])
                nc.gpsimd.dma_start(bias_sb[:], bias1[:])
                nc.gpsimd.dma_start(ident_sb[:], ident[:])
                nc.gpsimd.dma_start(nfb_sb[:], nfb[:])

            acc = acc_pool.tile([128, NT, HID], f32)
            out_r = out[:].rearrange("(a p) c -> p a c", p=128)
            for g in range(G):
                e0 = g * GRAN
                gw = min(GRAN, n_chunks * 128 - e0)
                xs_t = xt_pool.tile([128, 2, GRAN], f16, tag="xs")
                nsplit = 4 if g == 0 else 2
                H = gw // nsplit
                xengs = (nc.sync, nc.scalar, nc.scalar, nc.sync,
                         nc.gpsimd, nc.sync, nc.scalar, nc.gpsimd)
                for ih in range(2):
                    for h in range(nsplit):
                        xengs[nsplit * ih + h].dma_start(
                            xs_t[:, ih, h * H:(h + 1) * H],
                            xsrc[:, ih, e0 + h * H:e0 + (h + 1) * H])
                efr = efr_pool.tile([128, 2, GRAN], f16, tag="efr")
                ea = eft[:]
                engs_map = {"a": nc.scalar, "s": nc.sync, "p": nc.gpsimd}
                engs = tuple(engs_map[ch] for ch in
                             os.environ.get("EFR_ENGS", "aspsapss"))
                for g2 in range(2):
                    for h in range(nsplit):
                        # efrep[16*dd+iw, e] = eft[8*g2+dd, e0+e]
                        rep = bass.AP(ea.tensor,
                                      ea.offset + 8 * g2 * E_pad + e0 + h * H,
                                      [[E_pad, 8], [0, 16], [1, H]])
                        engs[nsplit * g2 + h].dma_start(
                            efr[:, g2, h * H:(h + 1) * H], rep)

                sel_t = sel_pool.tile([128, CH, 128], f16, tag="sel")
                nc.gpsimd.dma_start(
                    sel_t[:, 0:gw // 128, :],
                    selh[:, e0:e0 + gw]
                    .rearrange("p (c n) -> p c n", n=128))

                if g == 0:
                    emit_consts()
                y_t = y_pool.tile([128, 4, GRAN], f16, tag="y")
                with nc.allow_low_precision("f16 products, tol loose"):
                    nY = 2 if g == 0 else 1
                    HY = gw // nY
                    for g2 in range(2):
                        for hy in range(nY):
                            ef1 = efr[:, g2, hy * HY:(hy + 1) * HY]
                            ef_bc = bass.AP(ef1.tensor, ef1.offset,
                                            [ef1.ap[0], [0, 2], [1, HY]])
                            nc.vector.tensor_tensor(
                                out=y_t[:, 2 * g2:2 * g2 + 2,
                                        hy * HY:(hy + 1) * HY],
                                in0=xs_t[:, :, hy * HY:(hy + 1) * HY],
                                in1=ef_bc, op=mybir.AluOpType.mult)

                for cb in range(0, CH, MB):
                    cis = [ci for ci in range(cb, min(cb + MB, CH))
                           if g * CH + ci < n_chunks]
                    if not cis:
                        break
                    nb = len(cis)
                    msg_ps = mps_pool.tile([128, MB, HID], f32, tag="msgp")
                    for j, ci in enumerate(cis):
                        sl = slice(128 * ci, 128 * ci + 128)
                        for t in range(4):
                            nc.tensor.matmul(
                                msg_ps[:, j, :], y_t[:, t, sl],
                                wbig_sb[:, 32 * t:32 * t + 32],
                                start=(t == 0), stop=False)
                        nc.tensor.matmul(msg_ps[:, j, :], xs_t[0:16, 0, sl],
                                         bew_sb[:, 0, :], start=False,
                                         stop=False)
                        nc.tensor.matmul(msg_ps[:, j, :], xs_t[0:16, 1, sl],
                                         bew_sb[:, 1, :], start=False,
                                         stop=True)
                    msg_sb = msg_pool.tile([128, MB, HID], f16, tag="msgs")
                    with nc.allow_low_precision("accumulated in f32 PSUM"):
                        nc.scalar.copy(msg_sb[:, 0:nb, :], msg_ps[:, 0:nb, :])
                    for j, ci in enumerate(cis):
                        c = g * CH + ci
                        a, k = c // CPT, c % CPT
                        if k == 0:
                            nc.tensor.matmul(acc[:, a, :], ident_sb[:],
                                             nfb_sb[:, a, :],
                                             start=True, stop=False)
                            nc.tensor.matmul(acc[:, a, :], ones_sb[:],
                                             bias_sb[:], start=False,
                                             stop=False)
                        nc.tensor.matmul(acc[:, a, :], sel_t[:, ci, :],
                                         msg_sb[:, j, :],
                                         start=False, stop=(k == CPT - 1))
                        if k == CPT - 1 and (a + 1) % OUT_BLK == 0:
                            b0 = a + 1 - OUT_BLK
                            ob = ob_pool.tile([128, OUT_BLK, HID], f16,
                                              tag="ob")
                            with nc.allow_low_precision("f16 out staging"):
                                nc.scalar.copy(ob[:], acc[:, b0:a + 1, :])
                            nc.sync.dma_start(out_r[:, b0:a + 1, :], ob[:])

            if n_chunks == NT * CPT:
                b0 = (NT // OUT_BLK) * OUT_BLK
                if b0 < NT:
                    ob = ob_pool.tile([128, NT - b0, HID], f16, name="obt")
                    with nc.allow_low_precision("f16 out staging"):
                        nc.scalar.copy(ob[:], acc[:, b0:NT, :])
                    nc.sync.dma_start(out_r[:, b0:NT, :], ob[:])
    nc.compile()
    return nc


_CACHE = {}


def kernel(nf, initial_ef, src, dst, We, be, bias):
    in_maps, CPT, E_pad, perms = _prep(nf, initial_ef, src, dst, We, be, bias)
    key = (CPT, E_pad)
    if key not in _CACHE:
        _CACHE[key] = build_nc(CPT, E_pad)
    nc = _CACHE[key]

    from concourse.bass_utils import run_bass_kernel_spmd
    res = run_bass_kernel_spmd(nc, in_maps, core_ids=list(range(NCORES)))
    outs = [r["out"][perms[c], :HID] for c, r in enumerate(res.results)]
    return np.ascontiguousarray(np.concatenate(outs, axis=0).astype(np.float32))
